# revision 53
# baseline (speedup 1.0000x reference)
"""MiniAttentionBlock (LayerNorm -> causal MHA -> out-proj + residual) on 8 trn2 cores.

Sharding: core i handles batch b=i//2, head-group g=i%2 (4 heads = 512 features).
Each core returns a partial [T, H] = attnout(4 heads) @ Wo[:, slice].T  (no residual);
the host sums the two partials per batch and adds the residual x.

All on-device data is bf16 (PSUM accumulation stays f32); inputs are quantized
to bf16 on the host.  Measured rel err vs the f32 reference: ~4.4e-3.

Key trick: QKV matmuls run on RAW x (no normalized-x pass exists at all) --
  q[m,t] = rstd[t] * ( sum_f W[m,f] x[f,t] + negws[m]*mu[t] + bias[m]*std[t] )
so the rank-2 LayerNorm correction enters as a K=2 matmul with rows [mu; std]
and the rstd scaling happens in the PSUM->SBUF copy-out multiply (DVE).  This
removes the stats->QKV serialization: PE starts QKV as soon as x lands.

Engine balance:
  PE    - all matmuls (stats reduces, QKV, scores, AV, denom reduce for the
          final head, rstd/rdenom broadcasts, out-proj)
  ACT   - squares, exp over kt-paired [128,1024] score tiles, rstd_b/rb copies
  DVE   - stats chains, QKV copy-out scaling, denom adds (bf16 2x), at-norm
          muls, y copies
  Pool  - causal-mask muls (gpsimd cannot touch PSUM), DMA issue

Causal masking: only the [128,128] diagonal block of each diagonal k-tile is
partial; it is masked by multiplying with a precomputed triangular bf16 mask.
Fully-masked columns are never computed (scores/exp/AV/denom column-trimmed)
and the stale region of the pt tile is never read.

PSUM (8 banks): psS 2x[128,2,512] scores + psQ 2x[128,512] (QKV groups,
out-proj, norm: denom row + rb broadcast share one tile) + psAV 2x[128,512]
(stats reduces + AV accumulators).  At lead-in the idle psS banks carry QKV
groups pairwise for extra depth.
Schedule: x/weight DMAs (wq first), stats(0..3), qkv(0,1 via psS), then
{attn(k), qkv(k+2), outproj(k-1)}; scores diag-first with AV lagged 3 pairs,
normalize lagged one head; softmax denominator via PE ones-matmul for the
last head so the kernel tail is short.
"""

import numpy as np

H = 1024
T = 2048
B = 4
NCORES = 8
D = 128          # head dim
HPC = 4          # heads per core
F = HPC * D      # 512 out features per core
NC = H // 128    # 8 feature chunks
NT = T // 128    # 16 token k-tiles
NQ = T // 512    # 4 token quarters
SCALE = float(D) ** -0.5

_CACHED = {}


def _build_program():
    import concourse.bass as bass
    import concourse.tile as tile
    from concourse import bacc, mybir
    from concourse.bass import ts

    f32 = mybir.dt.float32
    bf16 = mybir.dt.bfloat16
    AL = mybir.AluOpType
    ACTF = mybir.ActivationFunctionType

    nc = bacc.Bacc("TRN2", target_bir_lowering=False, debug=False, num_devices=NCORES)

    xT = nc.dram_tensor("xT", [H, T], bf16, kind="ExternalInput").ap()
    wqT = nc.dram_tensor("wqT", [H, F], bf16, kind="ExternalInput").ap()
    wkT = nc.dram_tensor("wkT", [H, F], bf16, kind="ExternalInput").ap()
    wvT = nc.dram_tensor("wvT", [H, F], bf16, kind="ExternalInput").ap()
    woT = nc.dram_tensor("woT", [F, H], bf16, kind="ExternalInput").ap()
    auxc = nc.dram_tensor("auxc", [2, 3 * F], bf16, kind="ExternalInput").ap()
    cst = nc.dram_tensor("cst", [T], bf16, kind="ExternalInput").ap()
    out = nc.dram_tensor("out", [T, H], bf16, kind="ExternalOutput").ap()

    with tile.TileContext(nc) as tc:
        with tc.tile_pool(name="persist", bufs=1) as persist:
            ones_col = persist.tile([128, 1], bf16)
            nc.gpsimd.dma_start(
                out=ones_col, in_=cst[:128].rearrange("(p o) -> p o", o=1)
            )
            ones_row = persist.tile([1, 128], bf16)
            nc.gpsimd.dma_start(
                out=ones_row, in_=cst[:128].rearrange("(o f) -> o f", o=1)
            )
            zero_col = persist.tile([128, 1], f32)
            nc.vector.memset(zero_col, 0.0)
            eps_sb = persist.tile([1, 1], f32)
            nc.vector.memset(eps_sb, 1e-5)
            # causal [128,128] triangular mask: tri[p, q] = 1 if q >= p else 0
            tri = persist.tile([128, 128], bf16)
            nc.vector.memset(tri, 1.0)
            nc.gpsimd.affine_select(
                out=tri, in_=tri, compare_op=AL.is_ge, fill=0.0,
                base=0, channel_multiplier=-1, pattern=[[1, 128]],
            )
            # stt2: row0 = mu, row1 = std (written per-quarter by stats)
            stt2 = persist.tile([2, T], bf16)
            # selector rows to scatter [mu; std] onto partitions 0/1 via PE
            sel10 = persist.tile([1, 2], bf16)
            nc.vector.memset(sel10, 0.0)
            nc.vector.memset(sel10[0:1, 0:1], 1.0)
            sel01 = persist.tile([1, 2], bf16)
            nc.vector.memset(sel01, 0.0)
            nc.vector.memset(sel01[0:1, 1:2], 1.0)
            auxs = persist.tile([2, 3, F], bf16, tag="auxs")
            aq_sb = auxs[:, 0, :]
            ak_sb = auxs[:, 1, :]
            av_sb = auxs[:, 2, :]

            qT_all = persist.tile([128, HPC, T], bf16, tag="qT")
            kT_all = persist.tile([128, HPC, T], bf16, tag="kT")
            v_all = persist.tile([128, NT, F], bf16, tag="v")
            at_all = persist.tile([128, HPC, T], bf16, tag="at")
            xt = persist.tile([128, NC, T], bf16, tag="xt")
            rstd_b = persist.tile([128, T], bf16, tag="rstdb")
            rstd_c = persist.tile([128, NT], f32, tag="rstdc")
            wq_sb = persist.tile([128, NC, F], bf16, tag="wq")
            wk_sb = persist.tile([128, NC, F], bf16, tag="wk")
            wv_sb = persist.tile([128, NC, F], bf16, tag="wv")
            wo_sb = persist.tile([128, HPC, H], bf16, tag="wo")

            xT_r = xT.rearrange("(c p) t -> p c t", p=128)

            def load_x(tq, eng, cs):
                for c in cs:
                    eng.dma_start(
                        out=xt[:, c, ts(tq, 512)], in_=xT_r[:, c, ts(tq, 512)]
                    )

            # DMA schedule: weights early on the slow Pool queue (wq first so
            # raw QKV matmuls can start), bulk x on the HWDGE queue.
            SYNC_CS = (0, 2, 4, 5, 6, 7)
            POOL_CS = (1, 3)
            nc.gpsimd.dma_start(
                out=wq_sb, in_=wqT.rearrange("(c p) m -> p c m", p=128)
            )
            load_x(0, nc.sync, SYNC_CS)
            load_x(0, nc.gpsimd, POOL_CS)
            nc.gpsimd.dma_start(
                out=auxs, in_=auxc.rearrange("p (g f) -> p g f", g=3)
            )
            load_x(1, nc.sync, SYNC_CS)
            load_x(1, nc.gpsimd, POOL_CS)
            nc.sync.dma_start(out=wk_sb, in_=wkT.rearrange("(c p) m -> p c m", p=128))
            load_x(2, nc.sync, SYNC_CS)
            load_x(2, nc.gpsimd, POOL_CS)
            nc.sync.dma_start(out=wv_sb, in_=wvT.rearrange("(c p) m -> p c m", p=128))
            load_x(3, nc.sync, SYNC_CS)
            load_x(3, nc.gpsimd, POOL_CS)
            nc.gpsimd.dma_start(out=wo_sb, in_=woT.rearrange("(c p) n -> p c n", p=128))

            with (
                tc.tile_pool(name="psS", bufs=2, space="PSUM") as psS,
                tc.tile_pool(name="psQ", bufs=2, space="PSUM") as psQ,
                tc.tile_pool(name="psAV", bufs=2, space="PSUM") as psAV,
                tc.tile_pool(name="sqp", bufs=8) as sqp,
                tc.tile_pool(name="stp", bufs=3) as stp,
                tc.tile_pool(name="ptp", bufs=7) as ptp,
                tc.tile_pool(name="dnp", bufs=2) as dnp,
                tc.tile_pool(name="rdp", bufs=2) as rdp,
                tc.tile_pool(name="rbp", bufs=2) as rbp,
                tc.tile_pool(name="yp", bufs=4) as yp,
            ):

                def do_stats(tq):
                    sl = ts(tq, 512)
                    sq_ts = []
                    for c in range(NC):
                        sq_t = sqp.tile([128, 512], bf16, tag="sqt")
                        nc.scalar.activation(
                            sq_t, xt[:, c, sl], ACTF.Square, bias=zero_col
                        )
                        sq_ts.append(sq_t)
                    mean_t = psAV.tile([128, 512], f32, tag="av")
                    mean_ps = mean_t[0:1, :]
                    for c in range(NC):
                        nc.tensor.matmul(
                            mean_ps, ones_col, xt[:, c, sl],
                            start=(c == 0), stop=(c == NC - 1),
                        )
                    sqr_t = psAV.tile([128, 512], f32, tag="av")
                    sq_ps = sqr_t[0:1, :]
                    for c in range(NC):
                        nc.tensor.matmul(
                            sq_ps, ones_col, sq_ts[c],
                            start=(c == 0), stop=(c == NC - 1),
                        )
                    mu_sb = stp.tile([1, 512], bf16, tag="mu_sb")
                    nc.vector.tensor_scalar_mul(mu_sb, mean_ps, 1.0 / H)
                    mean_sb = stp.tile([1, 512], f32, tag="mean_sb")
                    nc.vector.tensor_copy(mean_sb, mean_ps)
                    # spre = mean^2 / H^2
                    spre = stp.tile([1, 512], f32, tag="spre")
                    nc.vector.scalar_tensor_tensor(
                        spre, mean_sb, 1.0 / (H * H), mean_sb,
                        op0=AL.mult, op1=AL.mult,
                    )
                    # var = meansq/H - spre
                    varr = stp.tile([1, 512], f32, tag="varr")
                    nc.vector.scalar_tensor_tensor(
                        varr, sq_ps, 1.0 / H, spre,
                        op0=AL.mult, op1=AL.subtract,
                    )
                    # std = sqrt(var + eps); stt2 row1 = std
                    std = stp.tile([1, 512], f32, tag="std")
                    nc.scalar.activation(std, varr, ACTF.Sqrt, bias=eps_sb)
                    std_bf = stp.tile([1, 512], bf16, tag="std_bf")
                    nc.vector.tensor_copy(std_bf, std)
                    # engines cannot write SBUF at partition offset 1, so build
                    # [mu; std] on PSUM partitions 0/1 via selector matmuls
                    st2_ps = psAV.tile([128, 512], f32, tag="av")
                    nc.tensor.matmul(
                        st2_ps[0:2, :], sel10, mu_sb, start=True, stop=False
                    )
                    nc.tensor.matmul(
                        st2_ps[0:2, :], sel01, std_bf, start=False, stop=True
                    )
                    nc.vector.tensor_copy(stt2[:, sl], st2_ps[0:2, :])
                    rstd = stp.tile([1, 512], bf16, tag="rstd")
                    with nc.allow_low_precision(reason="bf16 rstd"):
                        nc.vector.reciprocal(rstd, std)
                    # broadcast rstd along partitions (rows) and to columns
                    bc_t = psAV.tile([128, 512], f32, tag="av")
                    nc.tensor.matmul(bc_t, ones_row, rstd, start=True, stop=True)
                    nc.vector.tensor_copy(rstd_b[:, sl], bc_t)
                    rc_t = psAV.tile([128, 512], f32, tag="av")
                    for i in range(4):
                        nc.tensor.matmul(
                            rc_t[:, i:i + 1], rstd[0:1, ts(i, 128)],
                            ones_row[0:1, 0:1], start=True, stop=True,
                        )
                    nc.vector.tensor_copy(rstd_c[:, 4 * tq:4 * tq + 4], rc_t[:, 0:4])

                def emit_qkv_group(ps, g):
                    kind, w_sb, aux_sb, dst, idx, sl = g
                    if kind == "qk":
                        for c in range(NC):
                            nc.tensor.matmul(
                                ps, w_sb[:, c, ts(idx, 128)], xt[:, c, sl],
                                start=(c == 0), stop=False,
                            )
                        nc.tensor.matmul(
                            ps, aux_sb[:, ts(idx, 128)], stt2[:, sl],
                            start=False, stop=True,
                        )
                        nc.vector.tensor_mul(dst[:, idx, sl], ps, rstd_b[:, sl])
                    else:
                        # V: token-major, full 512 features per 128-token tile
                        tsl = ts(idx, 128)
                        for c in range(NC):
                            nc.tensor.matmul(
                                ps, xt[:, c, tsl], wv_sb[:, c, :],
                                start=(c == 0), stop=False,
                            )
                        nc.tensor.matmul(
                            ps, stt2[:, tsl], av_sb, start=False, stop=True
                        )
                        nc.vector.tensor_scalar_mul(
                            v_all[:, idx, :], ps, rstd_c[:, idx:idx + 1]
                        )

                def do_qkv(tq, lead=False):
                    sl = ts(tq, 512)
                    gs = []
                    for w_sb, aux_sb, dst in (
                        (wq_sb, aq_sb, qT_all),
                        (wk_sb, ak_sb, kT_all),
                    ):
                        for mi in range(HPC):
                            gs.append(("qk", w_sb, aux_sb, dst, mi, sl))
                    for ti in range(4 * tq, 4 * tq + 4):
                        gs.append(("v", None, None, None, ti, sl))

                    if lead:
                        # before attention starts, the psS banks are idle:
                        # run QKV groups through them pairwise for more depth
                        for a in range(0, len(gs), 2):
                            st = psS.tile([128, 2, 512], f32, tag="s")
                            emit_qkv_group(st[:, 0, :], gs[a])
                            emit_qkv_group(st[:, 1, :], gs[a + 1])
                    else:
                        for g in gs:
                            ps = psQ.tile([128, 512], f32, tag="a")
                            emit_qkv_group(ps, g)

                def emit_norm(h, av_t, dn, qsl, nt=None, rb_dve=False):
                    # denom reduce + reciprocal + bcast + normalize; dnr and rb
                    # share one PSUM tile (rb overwrites after recip reads row 0)
                    if nt is None:
                        nt = psQ.tile([128, 512], f32, tag="a")
                        nc.tensor.matmul(
                            nt[0:1, :], ones_col, dn, start=True, stop=True
                        )
                    rdenom = rdp.tile([1, 512], bf16, tag="rd")
                    with nc.allow_low_precision(reason="bf16 rdenom"):
                        nc.vector.reciprocal(rdenom, nt[0:1, :])
                    nc.tensor.matmul(nt, ones_row, rdenom, start=True, stop=True)
                    rb_sb = rbp.tile([128, 512], bf16, tag="rbs")
                    if rb_dve:
                        nc.vector.tensor_copy(rb_sb, nt)
                    else:
                        nc.scalar.activation(rb_sb, nt, ACTF.Copy)
                    nc.vector.tensor_mul(at_all[:, h, qsl], av_t, rb_sb)

                def do_attn(k):
                    qsl = ts(k, 512)
                    npair = 2 * k + 2
                    pend = [None]  # deferred normalize for head h-1
                    for h in range(HPC):
                        qh = qT_all[:, h, :]
                        kh = kT_all[:, h, :]
                        # last head of last quarter: accumulate the softmax
                        # denominator on PE (ones-matmul per tile) to shorten
                        # the end-of-kernel normalize chain
                        pe_denom = k == NQ - 1 and h == HPC - 1
                        av_t = psAV.tile([128, 512], f32, tag="av")
                        if pe_denom:
                            nt_h = psAV.tile([128, 512], f32, tag="av", name="nt_h")
                            dn = None
                        else:
                            nt_h = None
                            dn = dnp.tile([128, 512], bf16, tag="dn")
                        # diag pairs first so their exp+mask latency is hidden
                        order = [npair - 2, npair - 1] + list(range(npair - 2))
                        nord = len(order)
                        pts = {}
                        navq = [0]

                        def emit_av(j):
                            pj, ptj = pts.pop(j)
                            kt0 = 2 * pj
                            diag = pj >= npair - 2
                            for half in range(2):
                                cl = 128 * (2 * (pj - npair + 2) + half) if diag else 0
                                first = navq[0] == 0
                                last = j == nord - 1 and half == 1
                                nc.tensor.matmul(
                                    av_t[:, cl:512],
                                    v_all[:, kt0 + half, ts(h, 128)],
                                    ptj[:, half, cl:512],
                                    start=first, stop=last,
                                    skip_group_check=True,
                                )
                                navq[0] += 1
                                if pe_denom:
                                    nc.tensor.matmul(
                                        nt_h[0:1, cl:512], ones_col,
                                        ptj[:, half, cl:512],
                                        start=first, stop=last,
                                        skip_group_check=True,
                                    )
                                elif j == 0 and half == 0:
                                    nc.vector.tensor_copy(dn, ptj[:, half, :])
                                else:
                                    nc.vector.tensor_add(
                                        dn[:, cl:512], dn[:, cl:512],
                                        ptj[:, half, cl:512],
                                    )

                        for i, p in enumerate(order):
                            s_t = psS.tile([128, 2, 512], f32, tag="s")
                            pt = ptp.tile([128, 2, 512], bf16, tag="pt")
                            kt0 = 2 * p
                            if p >= npair - 2:
                                # diagonal pair: column-trimmed per half + mask
                                for half in range(2):
                                    jj = 2 * (p - npair + 2) + half
                                    cl = 128 * jj
                                    nc.tensor.matmul(
                                        s_t[:, half, cl:512],
                                        kh[:, ts(kt0 + half, 128)],
                                        qh[:, 512 * k + cl:512 * k + 512],
                                        start=True, stop=True,
                                    )
                                    nc.scalar.activation(
                                        pt[:, half, cl:512], s_t[:, half, cl:512],
                                        ACTF.Exp, bias=zero_col, scale=SCALE,
                                    )
                                    nc.gpsimd.tensor_mul(
                                        pt[:, half, cl:cl + 128],
                                        pt[:, half, cl:cl + 128], tri,
                                    )
                            else:
                                for half in range(2):
                                    nc.tensor.matmul(
                                        s_t[:, half, :],
                                        kh[:, ts(kt0 + half, 128)], qh[:, qsl],
                                        start=True, stop=True,
                                    )
                                nc.scalar.activation(
                                    pt, s_t, ACTF.Exp, bias=zero_col, scale=SCALE
                                )
                            pts[i] = (p, pt)
                            if i == 1 and pend[0] is not None:
                                emit_norm(*pend[0])
                                pend[0] = None
                            if i >= 4:
                                emit_av(i - 4)
                        for j in range(max(0, nord - 4), nord):
                            emit_av(j)
                        pend[0] = (h, av_t, dn, qsl, nt_h, True)
                    emit_norm(*pend[0])

                def do_outproj(k):
                    for ti in range(4 * k, 4 * k + 4):
                        tsl = ts(ti, 128)
                        for hc in range(2):
                            hsl = ts(hc, 512)
                            y_t = psQ.tile([128, 512], f32, tag="a")
                            for c in range(HPC):
                                nc.tensor.matmul(
                                    y_t, at_all[:, c, tsl], wo_sb[:, c, hsl],
                                    start=(c == 0), stop=(c == HPC - 1),
                                )
                            y_sb = yp.tile([128, 512], bf16, tag="ysb")
                            nc.vector.tensor_copy(y_sb, y_t)
                            nc.sync.dma_start(out=out[tsl, hsl], in_=y_sb)

                # ---- schedule --------------------------------------------------
                do_stats(0)
                do_stats(1)
                do_stats(2)
                do_stats(3)
                do_qkv(0, lead=True)
                do_qkv(1, lead=True)
                for k in range(NQ):
                    do_attn(k)
                    if k > 0:
                        do_outproj(k - 1)
                    if k + 2 < NQ:
                        do_qkv(k + 2)
                do_outproj(NQ - 1)

    nc.compile()
    return nc


def _get_program():
    if "nc" not in _CACHED:
        _CACHED["nc"] = _build_program()
    return _CACHED["nc"]


def _prep_core_inputs(x, gamma, beta, Wq, Wk, Wv, Wo, core):
    import ml_dtypes

    bf = ml_dtypes.bfloat16
    b, g = core // 2, core % 2
    gs = slice(g * F, (g + 1) * F)
    ins = {"xT": np.ascontiguousarray(x[b].T).astype(bf)}
    negs, biases = [], []
    for name, W in (("q", Wq), ("k", Wk), ("v", Wv)):
        W_eff = W[gs, :] * gamma[None, :]
        ins["w%sT" % name] = np.ascontiguousarray(W_eff.T).astype(bf)
        biases.append(W[gs, :] @ beta)
        negs.append(-W_eff.sum(axis=1))
    ins["auxc"] = np.stack(
        [np.concatenate(negs), np.concatenate(biases)]
    ).astype(bf)
    ins["woT"] = np.ascontiguousarray(Wo[:, gs].T).astype(bf)
    ins["cst"] = np.ones(T, bf)
    return ins


def kernel(x, gamma, beta, Wq, Wk, Wv, Wo, _trace=False):
    from concourse.bass_utils import run_bass_kernel_spmd

    x = np.asarray(x, dtype=np.float32)
    gamma = np.asarray(gamma, dtype=np.float32)
    beta = np.asarray(beta, dtype=np.float32)
    Wq, Wk = np.asarray(Wq, np.float32), np.asarray(Wk, np.float32)
    Wv, Wo = np.asarray(Wv, np.float32), np.asarray(Wo, np.float32)

    nc = _get_program()
    in_maps = [
        _prep_core_inputs(x, gamma, beta, Wq, Wk, Wv, Wo, i) for i in range(NCORES)
    ]
    res = run_bass_kernel_spmd(nc, in_maps, list(range(NCORES)), trace=_trace)
    _CACHED["last_result"] = res
    y = np.empty((B, T, H), np.float32)
    for b in range(B):
        y[b] = (
            np.asarray(res.results[2 * b]["out"], np.float32)
            + np.asarray(res.results[2 * b + 1]["out"], np.float32)
            + x[b]
        )
    return y
